# revision 11
# baseline (speedup 1.0000x reference)
"""CoarseMatching dual-softmax kernel for 8 Trainium2 NeuronCores.

Computes conf = softmax_L(sim) * softmax_S(sim) for sim = (f0*s)@(f1*s).T / T,
plus threshold/mutual-NN match extraction, matching reference.py semantics.

Sharding: core c -> (batch b = c//4, L-rows [j*1200,(j+1)*1200) for j = c%4).
Two SPMD launches:
  K1: per-core row sums (free via ACT accum) + column partial sums of exp(sim).
  K2: conf = exp(2*sim - ln(rowsum) - ln(colsum)) streamed to HBM.
Cross-core reduction (colsum over the 4 row-shards of a batch) happens on host
between launches. conf > THR detection is exact: ACT accum gives per-row block
sums of conf; conf>0 so any element > THR forces its block sum > THR. The
(practically unreachable) fallback computes the mask from conf in numpy with
reference semantics.
"""
import os
import subprocess
import sys
import tempfile

import numpy as np

sys.path.insert(0, "/opt/trn_rl_repo")


def _axon_available():
    """True if this process can reach the axon-tunneled NeuronCores."""
    try:
        import jax

        return any(d.platform in ("axon", "neuron") for d in jax.devices())
    except Exception:
        return False


def _import_bass():
    global bass, mybir, tile, run_bass_kernel_spmd, F32, F32R, AF
    import concourse.bass as bass
    import concourse.mybir as mybir
    import concourse.tile as tile
    from concourse.bass_utils import run_bass_kernel_spmd

    F32 = mybir.dt.float32
    F32R = mybir.dt.float32r
    AF = mybir.ActivationFunctionType

# problem constants (hardcoded per harness contract)
N, L, S, C = 2, 4800, 4800, 256
TEMPERATURE = 0.1
THR = 0.2
BORDER_RM = 2

NCORES = 8
SHARD = L // 4          # 1200 rows per core
LT = 128                # partition tile
N_LT = (SHARD + LT - 1) // LT   # 10 (9 full + 48)
# S-axis tiles: PSUM-bank-aligned (bank = 512 fp32). Each matmul chunk must
# stay inside one bank, so tiles are split at 512-element boundaries.
ST_SIZES = [1024, 1024, 1024, 1024, 704]
ST_OFFS = [0, 1024, 2048, 3072, 4096]
N_ST = len(ST_SIZES)
STMAX = 1024

def _chunks(width):
    out, o = [], 0
    while o < width:
        c = min(512, width - o)
        out.append((o, c))
        o += c
    return out

_MAXW = 1  # this toolchain's walrus rejects >1 sync wait on most instructions


def _split_excess_waits(nc):
    """Move excess per-instruction sem waits onto same-engine no-op carriers."""
    cnt = 0
    for f in nc.m.functions:
        for bb in f.blocks:
            il = bb.instructions
            i = 0
            while i < len(il):
                ins = il[i]
                si = ins.sync_info
                lim = 0 if isinstance(ins, mybir.InstMatmult) else _MAXW
                if si is not None and si.on_wait and len(si.on_wait) > lim:
                    waits = list(si.on_wait)
                    si.on_wait = waits[:lim]
                    excess = waits[lim:]
                    for j in range(0, len(excess), _MAXW):
                        cnt += 1
                        nop = mybir.InstNoOp(name=f"waitcarrier_{cnt}", ins=[], outs=[])
                        nop.engine = ins.engine
                        nop.sync_info = mybir.SyncInfo(
                            on_wait=excess[j : j + _MAXW], on_update=[]
                        )
                        il.insert(i, nop)
                        i += 1
                i += 1
    return cnt


def _lt_sizes():
    sizes = []
    for lt in range(N_LT):
        sizes.append(min(LT, SHARD - lt * LT))
    return sizes


def _build_k1():
    """Stats pass: rowsum[SHARD], colsum_part[S] of exp(sim) over local rows."""
    nc = bass.Bass("TRN2")
    a = nc.dram_tensor("a", [C // 128, 128, SHARD], F32R, kind="ExternalInput")
    b = nc.dram_tensor("b", [C // 128, 128, S], F32R, kind="ExternalInput")
    ones128 = nc.dram_tensor("ones128", [128, 1], F32R, kind="ExternalInput")
    rs_out = nc.dram_tensor("rs_out", [128, N_LT * N_ST], F32, kind="ExternalOutput")
    cs_out = nc.dram_tensor("cs_out", [1, S], F32, kind="ExternalOutput")

    lts = _lt_sizes()
    with tile.TileContext(nc) as tc:
        with tc.tile_pool(name="big", bufs=1) as big, \
             tc.tile_pool(name="work", bufs=3) as work, \
             tc.tile_pool(name="ps", bufs=2, space="PSUM") as ps, \
             tc.tile_pool(name="csps", bufs=2, space="PSUM") as csps:
            a_sb = big.tile([128, 2, SHARD], F32R)
            b_sb = big.tile([128, 2, S], F32R)
            ones_sb = big.tile([128, 1], F32R)
            rs_sb = big.tile([128, N_LT * N_ST], F32)
            cs_sb = big.tile([1, S], F32)
            nc.sync.dma_start(out=a_sb[:, 0, :], in_=a[0])
            nc.sync.dma_start(out=a_sb[:, 1, :], in_=a[1])
            nc.sync.dma_start(out=b_sb[:, 0, :], in_=b[0])
            nc.sync.dma_start(out=b_sb[:, 1, :], in_=b[1])
            nc.sync.dma_start(out=ones_sb[:], in_=ones128[:])
            nc.vector.memset(rs_sb[:], 0.0)
            nc.vector.memset(cs_sb[:], 0.0)

            for st in range(N_ST):
                s0, stw = ST_OFFS[st], ST_SIZES[st]
                cs_ps = csps.tile([1, STMAX], F32, tag="cs")
                for lt in range(N_LT):
                    pl = lts[lt]
                    l0 = lt * LT
                    sim_ps = ps.tile([128, STMAX], F32, tag="sim")
                    for o, w in _chunks(stw):
                        hs = slice(s0 + o, s0 + o + w)
                        ph = slice(o, o + w)
                        nc.tensor.matmul(
                            sim_ps[:pl, ph],
                            a_sb[:, 0, l0 : l0 + pl],
                            b_sb[:, 0, hs],
                            start=True, stop=False,
                        )
                        nc.tensor.matmul(
                            sim_ps[:pl, ph],
                            a_sb[:, 1, l0 : l0 + pl],
                            b_sb[:, 1, hs],
                            start=False, stop=True,
                        )
                    t_sb = work.tile([128, STMAX], F32R, tag="t")
                    idx = lt * N_ST + st
                    nc.scalar.activation(
                        t_sb[:pl, :stw], sim_ps[:pl, :stw], AF.Exp, scale=1.0,
                    )
                    nc.vector.reduce_sum(
                        rs_sb[:pl, idx : idx + 1],
                        t_sb[:pl, :stw].bitcast(F32),
                        axis=mybir.AxisListType.X,
                    )
                    for o, w in _chunks(stw):
                        ph = slice(o, o + w)
                        nc.tensor.matmul(
                            cs_ps[:1, ph],
                            ones_sb[:pl, :],
                            t_sb[:pl, ph],
                            start=(lt == 0), stop=(lt == N_LT - 1),
                        )
                nc.vector.tensor_copy(cs_sb[:1, s0 : s0 + stw], cs_ps[:1, :stw])

            nc.sync.dma_start(out=rs_out[:, :], in_=rs_sb[:])
            nc.sync.dma_start(out=cs_out[:, :], in_=cs_sb[:])

    _split_excess_waits(nc)
    return nc


def _build_k2():
    """Output pass: conf tiles = exp(2*sim - lnrs - lncs), plus block row sums."""
    nc = bass.Bass("TRN2")
    a = nc.dram_tensor("a", [C // 128, 128, SHARD], F32R, kind="ExternalInput")
    aon = nc.dram_tensor("aon", [2, SHARD], F32R, kind="ExternalInput")   # ones rows
    b = nc.dram_tensor("b", [C // 128, 128, S], F32R, kind="ExternalInput")
    bfold = nc.dram_tensor("bfold", [2, S], F32R, kind="ExternalInput")   # -lncs/2 hi/lo
    rbias = nc.dram_tensor("rbias", [128, N_LT], F32, kind="ExternalInput")  # -lnrs
    conf_out = nc.dram_tensor("conf_out", [SHARD, S], F32, kind="ExternalOutput")
    racc_out = nc.dram_tensor("racc_out", [128, N_LT * N_ST], F32, kind="ExternalOutput")

    lts = _lt_sizes()
    with tile.TileContext(nc) as tc:
        with tc.tile_pool(name="big", bufs=1) as big, \
             tc.tile_pool(name="work", bufs=4) as work, \
             tc.tile_pool(name="ps", bufs=3, space="PSUM") as ps:
            a_sb = big.tile([128, 2, SHARD], F32R)
            aon_sb = big.tile([2, SHARD], F32R)
            b_sb = big.tile([128, 2, S], F32R)
            bf_sb = big.tile([2, S], F32R)
            rb_sb = big.tile([128, N_LT], F32)
            racc_sb = big.tile([128, N_LT * N_ST], F32)
            nc.sync.dma_start(out=a_sb[:, 0, :], in_=a[0])
            nc.sync.dma_start(out=a_sb[:, 1, :], in_=a[1])
            nc.sync.dma_start(out=aon_sb[:], in_=aon[:])
            nc.sync.dma_start(out=b_sb[:, 0, :], in_=b[0])
            nc.sync.dma_start(out=b_sb[:, 1, :], in_=b[1])
            nc.sync.dma_start(out=bf_sb[:], in_=bfold[:])
            nc.sync.dma_start(out=rb_sb[:], in_=rbias[:])
            nc.vector.memset(racc_sb[:], 0.0)

            for lt in range(N_LT):
                pl = lts[lt]
                l0 = lt * LT
                for st in range(N_ST):
                    s0, stw = ST_OFFS[st], ST_SIZES[st]
                    sim_ps = ps.tile([128, STMAX], F32, tag="sim")
                    for o, w in _chunks(stw):
                        hs = slice(s0 + o, s0 + o + w)
                        ph = slice(o, o + w)
                        nc.tensor.matmul(
                            sim_ps[:pl, ph],
                            a_sb[:, 0, l0 : l0 + pl],
                            b_sb[:, 0, hs],
                            start=True, stop=False,
                        )
                        nc.tensor.matmul(
                            sim_ps[:pl, ph],
                            a_sb[:, 1, l0 : l0 + pl],
                            b_sb[:, 1, hs],
                            start=False, stop=False,
                        )
                        # fold in -lncs/2 via two extra contraction rows (hi+lo)
                        nc.tensor.matmul(
                            sim_ps[:pl, ph],
                            aon_sb[:, l0 : l0 + pl],
                            bf_sb[:, hs],
                            start=False, stop=True,
                        )
                    conf_sb = work.tile([128, STMAX], F32, tag="conf")
                    idx = lt * N_ST + st
                    nc.scalar.activation(
                        conf_sb[:pl, :stw], sim_ps[:pl, :stw], AF.Exp,
                        bias=rb_sb[:pl, lt : lt + 1], scale=2.0,
                    )
                    nc.vector.reduce_sum(
                        racc_sb[:pl, idx : idx + 1],
                        conf_sb[:pl, :stw],
                        axis=mybir.AxisListType.X,
                    )
                    nc.sync.dma_start(
                        out=conf_out[l0 : l0 + pl, s0 : s0 + stw],
                        in_=conf_sb[:pl, :stw],
                    )

            nc.sync.dma_start(out=racc_out[:, :], in_=racc_sb[:])

    _split_excess_waits(nc)
    return nc


_CACHE = {}


def _get_kernels():
    if "k1" not in _CACHE:
        _import_bass()
        _CACHE["k1"] = _build_k1()
        _CACHE["k2"] = _build_k2()
    return _CACHE["k1"], _CACHE["k2"]


def _trunc10(x):
    """Truncate fp32 mantissa to 10 explicit bits (exactly f32r-representable)."""
    xi = np.ascontiguousarray(x, dtype=np.float32).view(np.uint32)
    xi = xi & np.uint32(0xFFFFE000)
    return xi.view(np.float32)


def _border_valid(h, w, bd):
    vh = np.arange(h) >= bd
    vw = np.arange(w) >= bd
    return (vh[:, None] & vw[None, :]).reshape(-1)


def _unpack_cols(arr, n_lt, lts):
    """[128, n_lt] column-per-ltile layout -> flat [sum(lts)]."""
    return np.concatenate([arr[: lts[lt], lt] for lt in range(n_lt)])


def _kernel_device(feat_c0, feat_c1, h0c, w0c, h1c, w1c):
    """Full computation; must run in a process with axon devices visible."""
    k1, k2 = _get_kernels()
    lts = _lt_sizes()

    # fold feature scale and temperature into the operands:
    # sim/T = (f0*k) @ (f1*k).T with k = C**-0.5 / sqrt(T)
    k = (C ** -0.5) / np.sqrt(np.float32(TEMPERATURE))
    f0t = np.ascontiguousarray(
        (feat_c0 * k).astype(np.float32).transpose(0, 2, 1)
    )  # [N, C, L]
    f1t = np.ascontiguousarray(
        (feat_c1 * k).astype(np.float32).transpose(0, 2, 1)
    )  # [N, C, S]

    a_maps = []
    for c in range(NCORES):
        bidx, j = divmod(c, 4)
        ash = f0t[bidx, :, j * SHARD : (j + 1) * SHARD]  # [C, SHARD]
        a_maps.append(np.ascontiguousarray(ash.reshape(2, 128, SHARD)))
    b_arrs = [np.ascontiguousarray(f1t[bidx].reshape(2, 128, S)) for bidx in range(N)]
    ones128 = np.ones((128, 1), np.float32)

    in_maps1 = [
        {"a": a_maps[c], "b": b_arrs[c // 4], "ones128": ones128}
        for c in range(NCORES)
    ]
    r1 = run_bass_kernel_spmd(k1, in_maps1, core_ids=list(range(NCORES))).results

    # host combine: rowsum per core (exclusive rows), colsum across the 4 shards
    rowsum = np.empty((N, L), np.float64)
    for c in range(NCORES):
        bidx, j = divmod(c, 4)
        rs = r1[c]["rs_out"].astype(np.float64).reshape(128, N_LT, N_ST).sum(axis=2)
        rowsum[bidx, j * SHARD : (j + 1) * SHARD] = _unpack_cols(rs, N_LT, lts)
    colsum = np.zeros((N, S), np.float64)
    for c in range(NCORES):
        bidx = c // 4
        colsum[bidx] += r1[c]["cs_out"][0].astype(np.float64)

    lnrs = np.log(rowsum)  # [N, L]
    lncs = np.log(colsum)  # [N, S]

    # K2 inputs
    neg_half_lncs = (-0.5 * lncs).astype(np.float32)  # [N, S]
    hi = _trunc10(neg_half_lncs)
    lo = (neg_half_lncs.astype(np.float64) - hi).astype(np.float32)
    bfolds = [np.ascontiguousarray(np.stack([hi[bidx], lo[bidx]])) for bidx in range(N)]
    aon = np.ones((2, SHARD), np.float32)

    in_maps2 = []
    for c in range(NCORES):
        bidx, j = divmod(c, 4)
        nlr = (-lnrs[bidx, j * SHARD : (j + 1) * SHARD]).astype(np.float32)
        rb = np.zeros((128, N_LT), np.float32)
        for lt in range(N_LT):
            rb[: lts[lt], lt] = nlr[lt * LT : lt * LT + lts[lt]]
        in_maps2.append(
            {
                "a": a_maps[c],
                "aon": aon,
                "b": b_arrs[bidx],
                "bfold": bfolds[bidx],
                "rbias": rb,
            }
        )
    r2 = run_bass_kernel_spmd(k2, in_maps2, core_ids=list(range(NCORES))).results

    conf = np.empty((N, L, S), np.float32)
    blocksum_max = 0.0
    for c in range(NCORES):
        bidx, j = divmod(c, 4)
        conf[bidx, j * SHARD : (j + 1) * SHARD, :] = r2[c]["conf_out"]
        racc = r2[c]["racc_out"].reshape(128, N_LT * N_ST)
        for lt in range(N_LT):
            m = racc[: lts[lt], lt * N_ST : (lt + 1) * N_ST]
            if m.size:
                blocksum_max = max(blocksum_max, float(m.max()))

    if blocksum_max <= THR:
        # conf > 0 everywhere, so every element < its block sum <= THR:
        # threshold mask is all-False -> trivial match outputs.
        match_mask = np.zeros((N, L), dtype=bool)
        j_ids = np.zeros((N, L), dtype=np.int32)
        mconf = np.zeros((N, L), dtype=np.float32)
    else:
        mask = conf > THR
        valid0 = _border_valid(h0c, w0c, BORDER_RM)
        valid1 = _border_valid(h1c, w1c, BORDER_RM)
        mask = mask & valid0[None, :, None] & valid1[None, None, :]
        mask = (
            mask
            & (conf == conf.max(axis=2, keepdims=True))
            & (conf == conf.max(axis=1, keepdims=True))
        )
        j_ids = np.argmax(mask, axis=2).astype(np.int32)
        match_mask = np.any(mask, axis=2)
        mconf = np.take_along_axis(conf, j_ids[..., None].astype(np.int64), axis=2)[
            ..., 0
        ]
        mconf = np.where(match_mask, mconf, 0.0).astype(np.float32)

    return conf, match_mask, j_ids, mconf


def kernel(feat_c0, feat_c1, h0c, w0c, h1c, w1c):
    feat_c0 = np.asarray(feat_c0, dtype=np.float32)
    feat_c1 = np.asarray(feat_c1, dtype=np.float32)
    h0c, w0c, h1c, w1c = int(h0c), int(w0c), int(h1c), int(w1c)
    assert feat_c0.shape == (N, L, C) and feat_c1.shape == (N, S, C)

    if _axon_available():
        return _kernel_device(feat_c0, feat_c1, h0c, w0c, h1c, w1c)

    # The calling process has jax pinned away from the axon platform (e.g.
    # JAX_PLATFORMS=cpu). Run the device work in a clean subprocess.
    with tempfile.TemporaryDirectory() as td:
        in_path = os.path.join(td, "in.npz")
        out_path = os.path.join(td, "out.npz")
        np.savez(in_path, feat_c0=feat_c0, feat_c1=feat_c1,
                 h0c=h0c, w0c=w0c, h1c=h1c, w1c=w1c)
        env = dict(os.environ)
        env.pop("JAX_PLATFORMS", None)
        subprocess.run(
            [sys.executable, os.path.abspath(__file__), "--device-worker",
             in_path, out_path],
            check=True, env=env,
        )
        with np.load(out_path) as z:
            return (z["conf"], z["match_mask"], z["j_ids"], z["mconf"])


if __name__ == "__main__" and len(sys.argv) >= 4 and sys.argv[1] == "--device-worker":
    with np.load(sys.argv[2]) as z:
        _args = (z["feat_c0"], z["feat_c1"],
                 int(z["h0c"]), int(z["w0c"]), int(z["h1c"]), int(z["w1c"]))
    _conf, _mm, _ji, _mc = _kernel_device(*_args)
    np.savez(sys.argv[3], conf=_conf, match_mask=_mm, j_ids=_ji, mconf=_mc)


# revision 17
# speedup vs baseline: 33189.3505x; 33189.3505x over previous
"""CoarseMatching dual-softmax kernel for 8 Trainium2 NeuronCores.

Computes conf = softmax_L(sim) * softmax_S(sim) for sim = (f0*s)@(f1*s).T / T,
plus threshold/mutual-NN match extraction, matching reference.py semantics.

Sharding: core c -> (batch b = c//4, L-rows [j*1200,(j+1)*1200) for j = c%4).
Two SPMD launches:
  K1: per-core row sums (free via ACT accum) + column partial sums of exp(sim).
  K2: conf = exp(2*sim - ln(rowsum) - ln(colsum)) streamed to HBM.
Cross-core reduction (colsum over the 4 row-shards of a batch) happens on host
between launches. conf > THR detection is exact: ACT accum gives per-row block
sums of conf; conf>0 so any element > THR forces its block sum > THR. The
(practically unreachable) fallback computes the mask from conf in numpy with
reference semantics.
"""
import os
import subprocess
import sys
import tempfile

import numpy as np

sys.path.insert(0, "/opt/trn_rl_repo")


def _axon_available():
    """True if this process can reach the axon-tunneled NeuronCores."""
    try:
        import jax

        return any(d.platform in ("axon", "neuron") for d in jax.devices())
    except Exception:
        return False


def _import_bass():
    global bass, mybir, tile, run_bass_kernel_spmd, F32, F32R, AF
    import concourse.bass as bass
    import concourse.mybir as mybir
    import concourse.tile as tile
    from concourse.bass_utils import run_bass_kernel_spmd

    F32 = mybir.dt.float32
    F32R = mybir.dt.float32r
    AF = mybir.ActivationFunctionType

# problem constants (hardcoded per harness contract)
N, L, S, C = 2, 4800, 4800, 256
TEMPERATURE = 0.1
THR = 0.2
BORDER_RM = 2

NCORES = 8
SHARD = L // 4          # 1200 rows per core
LT = 128                # partition tile
N_LT = (SHARD + LT - 1) // LT   # 10 (9 full + 48)
# S-axis tiles: PSUM-bank-aligned (bank = 512 fp32). Each matmul chunk must
# stay inside one bank, so tiles are split at 512-element boundaries.
ST_SIZES = [1024, 1024, 1024, 1024, 704]
ST_OFFS = [0, 1024, 2048, 3072, 4096]
N_ST = len(ST_SIZES)
STMAX = 1024

def _chunks(width):
    out, o = [], 0
    while o < width:
        c = min(512, width - o)
        out.append((o, c))
        o += c
    return out

_MAXW = 1  # this toolchain's walrus rejects >1 sync wait on most instructions


def _split_excess_waits(nc):
    """Move excess per-instruction sem waits onto same-engine no-op carriers."""
    if os.environ.get("KERNEL_NO_WAIT_SPLIT"):
        return 0
    cnt = 0
    for f in nc.m.functions:
        for bb in f.blocks:
            il = bb.instructions
            i = 0
            while i < len(il):
                ins = il[i]
                si = ins.sync_info
                lim = 0 if isinstance(ins, mybir.InstMatmult) else _MAXW
                if si is not None and si.on_wait and len(si.on_wait) > lim:
                    waits = list(si.on_wait)
                    si.on_wait = waits[:lim]
                    excess = waits[lim:]
                    for j in range(0, len(excess), _MAXW):
                        cnt += 1
                        nop = mybir.InstNoOp(name=f"waitcarrier_{cnt}", ins=[], outs=[])
                        nop.engine = ins.engine
                        nop.sync_info = mybir.SyncInfo(
                            on_wait=excess[j : j + _MAXW], on_update=[]
                        )
                        il.insert(i, nop)
                        i += 1
                i += 1
    return cnt


def _lt_sizes():
    sizes = []
    for lt in range(N_LT):
        sizes.append(min(LT, SHARD - lt * LT))
    return sizes


def _build_k1():
    """Stats pass: rowsum[SHARD], colsum_part[S] of exp(sim) over local rows."""
    nc = bass.Bass("TRN2")
    a = nc.dram_tensor("a", [C // 128, 128, SHARD], F32R, kind="ExternalInput")
    b = nc.dram_tensor("b", [C // 128, 128, S], F32R, kind="ExternalInput")
    ones128 = nc.dram_tensor("ones128", [128, 1], F32R, kind="ExternalInput")
    rs_out = nc.dram_tensor("rs_out", [128, N_LT * N_ST], F32, kind="ExternalOutput")
    cs_out = nc.dram_tensor("cs_out", [1, S], F32, kind="ExternalOutput")

    lts = _lt_sizes()
    with tile.TileContext(nc) as tc:
        with tc.tile_pool(name="big", bufs=1) as big, \
             tc.tile_pool(name="work", bufs=6) as work, \
             tc.tile_pool(name="ps", bufs=3, space="PSUM") as ps, \
             tc.tile_pool(name="csps", bufs=1, space="PSUM") as csps:
            a_sb = big.tile([128, 2, SHARD], F32R)
            b_sb = big.tile([128, 2, S], F32R)
            ones_sb = big.tile([128, 1], F32R)
            rs_sb = big.tile([128, N_LT * N_ST], F32)
            cs_sb = big.tile([1, S], F32)
            nc.sync.dma_start(out=a_sb[:, 0, :], in_=a[0])
            nc.sync.dma_start(out=a_sb[:, 1, :], in_=a[1])
            nc.sync.dma_start(out=b_sb[:, 0, :1024], in_=b[0][:, :1024])
            nc.sync.dma_start(out=b_sb[:, 1, :1024], in_=b[1][:, :1024])
            nc.sync.dma_start(out=b_sb[:, 0, 1024:], in_=b[0][:, 1024:])
            nc.sync.dma_start(out=b_sb[:, 1, 1024:], in_=b[1][:, 1024:])
            nc.sync.dma_start(out=ones_sb[:], in_=ones128[:])
            nc.vector.memset(rs_sb[:], 0.0)
            nc.vector.memset(cs_sb[:], 0.0)

            for st in range(N_ST):
                s0, stw = ST_OFFS[st], ST_SIZES[st]
                cs_ps = csps.tile([1, STMAX], F32, tag="cs")
                for lt in range(N_LT):
                    pl = lts[lt]
                    l0 = lt * LT
                    sim_ps = ps.tile([128, STMAX], F32, tag="sim")
                    for o, w in _chunks(stw):
                        hs = slice(s0 + o, s0 + o + w)
                        ph = slice(o, o + w)
                        nc.tensor.matmul(
                            sim_ps[:pl, ph],
                            a_sb[:, 0, l0 : l0 + pl],
                            b_sb[:, 0, hs],
                            start=True, stop=False,
                        )
                        nc.tensor.matmul(
                            sim_ps[:pl, ph],
                            a_sb[:, 1, l0 : l0 + pl],
                            b_sb[:, 1, hs],
                            start=False, stop=True,
                        )
                    t_sb = work.tile([128, STMAX], F32R, tag="t")
                    idx = lt * N_ST + st
                    nc.scalar.activation(
                        t_sb[:pl, :stw], sim_ps[:pl, :stw], AF.Exp, scale=1.0,
                    )
                    nc.vector.reduce_sum(
                        rs_sb[:pl, idx : idx + 1],
                        t_sb[:pl, :stw].bitcast(F32),
                        axis=mybir.AxisListType.X,
                    )
                    for o, w in _chunks(stw):
                        ph = slice(o, o + w)
                        nc.tensor.matmul(
                            cs_ps[:1, ph],
                            ones_sb[:pl, :],
                            t_sb[:pl, ph],
                            start=(lt == 0), stop=(lt == N_LT - 1),
                        )
                nc.vector.tensor_copy(cs_sb[:1, s0 : s0 + stw], cs_ps[:1, :stw])

            nc.sync.dma_start(out=rs_out[:, :], in_=rs_sb[:])
            nc.sync.dma_start(out=cs_out[:, :], in_=cs_sb[:])

    _split_excess_waits(nc)
    return nc


def _build_k2():
    """Output pass: conf tiles = exp(2*sim - lnrs - lncs), plus block row sums."""
    nc = bass.Bass("TRN2")
    a = nc.dram_tensor("a", [C // 128, 128, SHARD], F32R, kind="ExternalInput")
    aon = nc.dram_tensor("aon", [2, SHARD], F32R, kind="ExternalInput")   # ones rows
    b = nc.dram_tensor("b", [C // 128, 128, S], F32R, kind="ExternalInput")
    bfold = nc.dram_tensor("bfold", [2, S], F32R, kind="ExternalInput")   # -lncs/2 hi/lo
    rbias = nc.dram_tensor("rbias", [128, N_LT], F32, kind="ExternalInput")  # -lnrs
    conf_out = nc.dram_tensor("conf_out", [SHARD, S], F32, kind="ExternalOutput")

    lts = _lt_sizes()
    with tile.TileContext(nc) as tc:
        with tc.tile_pool(name="big", bufs=1) as big, \
             tc.tile_pool(name="work", bufs=6) as work, \
             tc.tile_pool(name="ps", bufs=3, space="PSUM") as ps:
            a_sb = big.tile([128, 2, SHARD], F32R)
            aon_sb = big.tile([2, SHARD], F32R)
            b_sb = big.tile([128, 2, S], F32R)
            bf_sb = big.tile([2, S], F32R)
            rb_sb = big.tile([128, N_LT], F32)
            nc.sync.dma_start(out=a_sb[:, 0, :], in_=a[0])
            nc.sync.dma_start(out=a_sb[:, 1, :], in_=a[1])
            nc.sync.dma_start(out=aon_sb[:], in_=aon[:])
            nc.sync.dma_start(out=b_sb[:, 0, :1024], in_=b[0][:, :1024])
            nc.sync.dma_start(out=b_sb[:, 1, :1024], in_=b[1][:, :1024])
            nc.sync.dma_start(out=b_sb[:, 0, 1024:], in_=b[0][:, 1024:])
            nc.sync.dma_start(out=b_sb[:, 1, 1024:], in_=b[1][:, 1024:])
            nc.sync.dma_start(out=bf_sb[:], in_=bfold[:])
            nc.sync.dma_start(out=rb_sb[:], in_=rbias[:])

            for lt in range(N_LT):
                pl = lts[lt]
                l0 = lt * LT
                for st in range(N_ST):
                    s0, stw = ST_OFFS[st], ST_SIZES[st]
                    sim_ps = ps.tile([128, STMAX], F32, tag="sim")
                    for o, w in _chunks(stw):
                        hs = slice(s0 + o, s0 + o + w)
                        ph = slice(o, o + w)
                        nc.tensor.matmul(
                            sim_ps[:pl, ph],
                            a_sb[:, 0, l0 : l0 + pl],
                            b_sb[:, 0, hs],
                            start=True, stop=False,
                        )
                        nc.tensor.matmul(
                            sim_ps[:pl, ph],
                            a_sb[:, 1, l0 : l0 + pl],
                            b_sb[:, 1, hs],
                            start=False, stop=False,
                        )
                        # fold in -lncs/2 via two extra contraction rows (hi+lo)
                        nc.tensor.matmul(
                            sim_ps[:pl, ph],
                            aon_sb[:, l0 : l0 + pl],
                            bf_sb[:, hs],
                            start=False, stop=True,
                        )
                    conf_sb = work.tile([128, STMAX], F32, tag="conf")
                    nc.scalar.activation(
                        conf_sb[:pl, :stw], sim_ps[:pl, :stw], AF.Exp,
                        bias=rb_sb[:pl, lt : lt + 1], scale=2.0,
                    )
                    nc.sync.dma_start(
                        out=conf_out[l0 : l0 + pl, s0 : s0 + stw],
                        in_=conf_sb[:pl, :stw],
                    )


    _split_excess_waits(nc)
    return nc


_CACHE = {}


def _get_kernels():
    if "k1" not in _CACHE:
        _import_bass()
        _CACHE["k1"] = _build_k1()
        _CACHE["k2"] = _build_k2()
    return _CACHE["k1"], _CACHE["k2"]


def _trunc10(x):
    """Truncate fp32 mantissa to 10 explicit bits (exactly f32r-representable)."""
    xi = np.ascontiguousarray(x, dtype=np.float32).view(np.uint32)
    xi = xi & np.uint32(0xFFFFE000)
    return xi.view(np.float32)


def _border_valid(h, w, bd):
    vh = np.arange(h) >= bd
    vw = np.arange(w) >= bd
    return (vh[:, None] & vw[None, :]).reshape(-1)


def _unpack_cols(arr, n_lt, lts):
    """[128, n_lt] column-per-ltile layout -> flat [sum(lts)]."""
    return np.concatenate([arr[: lts[lt], lt] for lt in range(n_lt)])


def _kernel_device(feat_c0, feat_c1, h0c, w0c, h1c, w1c):
    """Full computation; must run in a process with axon devices visible."""
    import time as _time
    _tp = [("start", _time.time())]
    k1, k2 = _get_kernels()
    _tp.append(("build", _time.time()))
    lts = _lt_sizes()

    # fold feature scale and temperature into the operands:
    # sim/T = (f0*k) @ (f1*k).T with k = C**-0.5 / sqrt(T)
    k = (C ** -0.5) / np.sqrt(np.float32(TEMPERATURE))
    f0t = np.ascontiguousarray(
        (feat_c0 * k).astype(np.float32).transpose(0, 2, 1)
    )  # [N, C, L]
    f1t = np.ascontiguousarray(
        (feat_c1 * k).astype(np.float32).transpose(0, 2, 1)
    )  # [N, C, S]

    a_maps = []
    for c in range(NCORES):
        bidx, j = divmod(c, 4)
        ash = f0t[bidx, :, j * SHARD : (j + 1) * SHARD]  # [C, SHARD]
        a_maps.append(np.ascontiguousarray(ash.reshape(2, 128, SHARD)))
    b_arrs = [np.ascontiguousarray(f1t[bidx].reshape(2, 128, S)) for bidx in range(N)]
    ones128 = np.ones((128, 1), np.float32)

    in_maps1 = [
        {"a": a_maps[c], "b": b_arrs[c // 4], "ones128": ones128}
        for c in range(NCORES)
    ]
    _tp.append(("prep1", _time.time()))
    r1 = run_bass_kernel_spmd(k1, in_maps1, core_ids=list(range(NCORES))).results
    _tp.append(("k1", _time.time()))

    # host combine: rowsum per core (exclusive rows), colsum across the 4 shards
    rowsum = np.empty((N, L), np.float64)
    for c in range(NCORES):
        bidx, j = divmod(c, 4)
        rs = r1[c]["rs_out"].astype(np.float64).reshape(128, N_LT, N_ST).sum(axis=2)
        rowsum[bidx, j * SHARD : (j + 1) * SHARD] = _unpack_cols(rs, N_LT, lts)
    colsum = np.zeros((N, S), np.float64)
    for c in range(NCORES):
        bidx = c // 4
        colsum[bidx] += r1[c]["cs_out"][0].astype(np.float64)

    lnrs = np.log(rowsum)  # [N, L]
    lncs = np.log(colsum)  # [N, S]

    # K2 inputs
    neg_half_lncs = (-0.5 * lncs).astype(np.float32)  # [N, S]
    hi = _trunc10(neg_half_lncs)
    lo = (neg_half_lncs.astype(np.float64) - hi).astype(np.float32)
    bfolds = [np.ascontiguousarray(np.stack([hi[bidx], lo[bidx]])) for bidx in range(N)]
    aon = np.ones((2, SHARD), np.float32)

    in_maps2 = []
    for c in range(NCORES):
        bidx, j = divmod(c, 4)
        nlr = (-lnrs[bidx, j * SHARD : (j + 1) * SHARD]).astype(np.float32)
        rb = np.zeros((128, N_LT), np.float32)
        for lt in range(N_LT):
            rb[: lts[lt], lt] = nlr[lt * LT : lt * LT + lts[lt]]
        in_maps2.append(
            {
                "a": a_maps[c],
                "aon": aon,
                "b": b_arrs[bidx],
                "bfold": bfolds[bidx],
                "rbias": rb,
            }
        )
    _tp.append(("prep2", _time.time()))
    r2 = run_bass_kernel_spmd(k2, in_maps2, core_ids=list(range(NCORES))).results
    _tp.append(("k2", _time.time()))

    conf = np.empty((N, L, S), np.float32)
    conf_max = 0.0
    for c in range(NCORES):
        bidx, j = divmod(c, 4)
        shard = r2[c]["conf_out"]
        conf[bidx, j * SHARD : (j + 1) * SHARD, :] = shard
        conf_max = max(conf_max, float(shard.max()))

    if conf_max <= THR:
        # threshold mask (conf > THR) is all-False -> trivial match outputs.
        match_mask = np.zeros((N, L), dtype=bool)
        j_ids = np.zeros((N, L), dtype=np.int32)
        mconf = np.zeros((N, L), dtype=np.float32)
    else:
        mask = conf > THR
        valid0 = _border_valid(h0c, w0c, BORDER_RM)
        valid1 = _border_valid(h1c, w1c, BORDER_RM)
        mask = mask & valid0[None, :, None] & valid1[None, None, :]
        mask = (
            mask
            & (conf == conf.max(axis=2, keepdims=True))
            & (conf == conf.max(axis=1, keepdims=True))
        )
        j_ids = np.argmax(mask, axis=2).astype(np.int32)
        match_mask = np.any(mask, axis=2)
        mconf = np.take_along_axis(conf, j_ids[..., None].astype(np.int64), axis=2)[
            ..., 0
        ]
        mconf = np.where(match_mask, mconf, 0.0).astype(np.float32)

    _tp.append(("assemble", _time.time()))
    if os.environ.get("KERNEL_TIMING"):
        for (n1, t1), (n2, t2) in zip(_tp, _tp[1:]):
            print(f"  phase {n2}: {t2 - t1:.3f}s", file=sys.stderr)
    return conf, match_mask, j_ids, mconf


def kernel(feat_c0, feat_c1, h0c, w0c, h1c, w1c):
    feat_c0 = np.asarray(feat_c0, dtype=np.float32)
    feat_c1 = np.asarray(feat_c1, dtype=np.float32)
    h0c, w0c, h1c, w1c = int(h0c), int(w0c), int(h1c), int(w1c)
    assert feat_c0.shape == (N, L, C) and feat_c1.shape == (N, S, C)

    if _axon_available():
        return _kernel_device(feat_c0, feat_c1, h0c, w0c, h1c, w1c)

    # The calling process has jax pinned away from the axon platform (e.g.
    # JAX_PLATFORMS=cpu). Run the device work in a clean subprocess.
    with tempfile.TemporaryDirectory() as td:
        in_path = os.path.join(td, "in.npz")
        out_path = os.path.join(td, "out.npz")
        np.savez(in_path, feat_c0=feat_c0, feat_c1=feat_c1,
                 h0c=h0c, w0c=w0c, h1c=h1c, w1c=w1c)
        env = dict(os.environ)
        env.pop("JAX_PLATFORMS", None)
        subprocess.run(
            [sys.executable, os.path.abspath(__file__), "--device-worker",
             in_path, out_path],
            check=True, env=env,
        )
        with np.load(out_path) as z:
            return (z["conf"], z["match_mask"], z["j_ids"], z["mconf"])


if __name__ == "__main__" and len(sys.argv) >= 4 and sys.argv[1] == "--device-worker":
    with np.load(sys.argv[2]) as z:
        _args = (z["feat_c0"], z["feat_c1"],
                 int(z["h0c"]), int(z["w0c"]), int(z["h1c"]), int(z["w1c"]))
    _conf, _mm, _ji, _mc = _kernel_device(*_args)
    np.savez(sys.argv[3], conf=_conf, match_mask=_mm, j_ids=_ji, mconf=_mc)


# revision 18
# speedup vs baseline: 34682.7764x; 1.0450x over previous
"""CoarseMatching dual-softmax kernel for 8 Trainium2 NeuronCores.

Computes conf = softmax_L(sim) * softmax_S(sim) for sim = (f0*s)@(f1*s).T / T,
plus threshold/mutual-NN match extraction, matching reference.py semantics.

Sharding: core c -> (batch b = c//4, L-rows [j*1200,(j+1)*1200) for j = c%4).
Two SPMD launches:
  K1: per-core row sums (free via ACT accum) + column partial sums of exp(sim).
  K2: conf = exp(2*sim - ln(rowsum) - ln(colsum)) streamed to HBM.
Cross-core reduction (colsum over the 4 row-shards of a batch) happens on host
between launches. conf > THR detection is exact: ACT accum gives per-row block
sums of conf; conf>0 so any element > THR forces its block sum > THR. The
(practically unreachable) fallback computes the mask from conf in numpy with
reference semantics.
"""
import os
import subprocess
import sys
import tempfile

import numpy as np

sys.path.insert(0, "/opt/trn_rl_repo")


def _axon_available():
    """True if this process can reach the axon-tunneled NeuronCores."""
    try:
        import jax

        return any(d.platform in ("axon", "neuron") for d in jax.devices())
    except Exception:
        return False


def _import_bass():
    global bass, mybir, tile, run_bass_kernel_spmd, F32, F32R, AF
    import concourse.bass as bass
    import concourse.mybir as mybir
    import concourse.tile as tile
    from concourse.bass_utils import run_bass_kernel_spmd

    F32 = mybir.dt.float32
    F32R = mybir.dt.float32r
    AF = mybir.ActivationFunctionType

# problem constants (hardcoded per harness contract)
N, L, S, C = 2, 4800, 4800, 256
TEMPERATURE = 0.1
THR = 0.2
BORDER_RM = 2

NCORES = 8
SHARD = L // 4          # 1200 rows per core
LT = 128                # partition tile
N_LT = (SHARD + LT - 1) // LT   # 10 (9 full + 48)
# S-axis tiles: PSUM-bank-aligned (bank = 512 fp32). Each matmul chunk must
# stay inside one bank, so tiles are split at 512-element boundaries.
ST_SIZES = [1024, 1024, 1024, 1024, 704]
ST_OFFS = [0, 1024, 2048, 3072, 4096]
N_ST = len(ST_SIZES)
STMAX = 1024

def _chunks(width):
    out, o = [], 0
    while o < width:
        c = min(512, width - o)
        out.append((o, c))
        o += c
    return out

_MAXW = 1  # this toolchain's walrus rejects >1 sync wait on most instructions


def _split_excess_waits(nc):
    """Move excess per-instruction sem waits onto same-engine no-op carriers."""
    if os.environ.get("KERNEL_NO_WAIT_SPLIT"):
        return 0
    cnt = 0
    for f in nc.m.functions:
        for bb in f.blocks:
            il = bb.instructions
            i = 0
            while i < len(il):
                ins = il[i]
                si = ins.sync_info
                lim = 0 if isinstance(ins, mybir.InstMatmult) else _MAXW
                if si is not None and si.on_wait and len(si.on_wait) > lim:
                    waits = list(si.on_wait)
                    si.on_wait = waits[:lim]
                    excess = waits[lim:]
                    for j in range(0, len(excess), _MAXW):
                        cnt += 1
                        nop = mybir.InstNoOp(name=f"waitcarrier_{cnt}", ins=[], outs=[])
                        nop.engine = ins.engine
                        nop.sync_info = mybir.SyncInfo(
                            on_wait=excess[j : j + _MAXW], on_update=[]
                        )
                        il.insert(i, nop)
                        i += 1
                i += 1
    return cnt


def _lt_sizes():
    sizes = []
    for lt in range(N_LT):
        sizes.append(min(LT, SHARD - lt * LT))
    return sizes


def _build_k1():
    """Stats pass: rowsum[SHARD], colsum_part[S] of exp(sim) over local rows."""
    nc = bass.Bass("TRN2")
    a = nc.dram_tensor("a", [C // 128, 128, SHARD], F32R, kind="ExternalInput")
    b = nc.dram_tensor("b", [C // 128, 128, S], F32R, kind="ExternalInput")
    ones128 = nc.dram_tensor("ones128", [128, 1], F32R, kind="ExternalInput")
    rs_out = nc.dram_tensor("rs_out", [128, N_LT * N_ST], F32, kind="ExternalOutput")
    cs_out = nc.dram_tensor("cs_out", [1, S], F32, kind="ExternalOutput")

    lts = _lt_sizes()
    with tile.TileContext(nc) as tc:
        with tc.tile_pool(name="big", bufs=1) as big, \
             tc.tile_pool(name="work", bufs=6) as work, \
             tc.tile_pool(name="ps", bufs=3, space="PSUM") as ps, \
             tc.tile_pool(name="csps", bufs=1, space="PSUM") as csps:
            a_sb = big.tile([128, 2, SHARD], F32R)
            b_sb = big.tile([128, 2, S], F32R)
            ones_sb = big.tile([128, 1], F32R)
            rs_sb = big.tile([128, N_LT * N_ST], F32)
            cs_sb = big.tile([1, S], F32)
            nc.sync.dma_start(out=a_sb[:, 0, :], in_=a[0])
            nc.sync.dma_start(out=a_sb[:, 1, :], in_=a[1])
            nc.sync.dma_start(out=b_sb[:, 0, :1024], in_=b[0][:, :1024])
            nc.sync.dma_start(out=b_sb[:, 1, :1024], in_=b[1][:, :1024])
            nc.sync.dma_start(out=b_sb[:, 0, 1024:], in_=b[0][:, 1024:])
            nc.sync.dma_start(out=b_sb[:, 1, 1024:], in_=b[1][:, 1024:])
            nc.sync.dma_start(out=ones_sb[:], in_=ones128[:])
            nc.vector.memset(rs_sb[:], 0.0)
            nc.vector.memset(cs_sb[:], 0.0)

            for st in range(N_ST):
                s0, stw = ST_OFFS[st], ST_SIZES[st]
                cs_ps = csps.tile([1, STMAX], F32, tag="cs")
                for lt in range(N_LT):
                    pl = lts[lt]
                    l0 = lt * LT
                    sim_ps = ps.tile([128, STMAX], F32, tag="sim")
                    for o, w in _chunks(stw):
                        hs = slice(s0 + o, s0 + o + w)
                        ph = slice(o, o + w)
                        nc.tensor.matmul(
                            sim_ps[:pl, ph],
                            a_sb[:, 0, l0 : l0 + pl],
                            b_sb[:, 0, hs],
                            start=True, stop=False,
                        )
                        nc.tensor.matmul(
                            sim_ps[:pl, ph],
                            a_sb[:, 1, l0 : l0 + pl],
                            b_sb[:, 1, hs],
                            start=False, stop=True,
                        )
                    t_sb = work.tile([128, STMAX], F32R, tag="t")
                    idx = lt * N_ST + st
                    nc.scalar.activation(
                        t_sb[:pl, :stw], sim_ps[:pl, :stw], AF.Exp, scale=1.0,
                    )
                    nc.vector.reduce_sum(
                        rs_sb[:pl, idx : idx + 1],
                        t_sb[:pl, :stw].bitcast(F32),
                        axis=mybir.AxisListType.X,
                    )
                    for o, w in _chunks(stw):
                        ph = slice(o, o + w)
                        nc.tensor.matmul(
                            cs_ps[:1, ph],
                            ones_sb[:pl, :],
                            t_sb[:pl, ph],
                            start=(lt == 0), stop=(lt == N_LT - 1),
                        )
                nc.vector.tensor_copy(cs_sb[:1, s0 : s0 + stw], cs_ps[:1, :stw])

            nc.sync.dma_start(out=rs_out[:, :], in_=rs_sb[:])
            nc.sync.dma_start(out=cs_out[:, :], in_=cs_sb[:])

    _split_excess_waits(nc)
    return nc


def _build_k2():
    """Output pass: conf tiles = exp(2*sim - lnrs - lncs), plus block row sums."""
    nc = bass.Bass("TRN2")
    a = nc.dram_tensor("a", [C // 128, 128, SHARD], F32R, kind="ExternalInput")
    aon = nc.dram_tensor("aon", [2, SHARD], F32R, kind="ExternalInput")   # ones rows
    b = nc.dram_tensor("b", [C // 128, 128, S], F32R, kind="ExternalInput")
    bfold = nc.dram_tensor("bfold", [2, S], F32R, kind="ExternalInput")   # -lncs/2 hi/lo
    rbias = nc.dram_tensor("rbias", [128, N_LT], F32, kind="ExternalInput")  # -lnrs
    conf_out = nc.dram_tensor("conf_out", [SHARD, S], F32, kind="ExternalOutput")

    lts = _lt_sizes()
    with tile.TileContext(nc) as tc:
        with tc.tile_pool(name="big", bufs=1) as big, \
             tc.tile_pool(name="work", bufs=6) as work, \
             tc.tile_pool(name="ps", bufs=3, space="PSUM") as ps:
            a_sb = big.tile([128, 2, SHARD], F32R)
            aon_sb = big.tile([2, SHARD], F32R)
            b_sb = big.tile([128, 2, S], F32R)
            bf_sb = big.tile([2, S], F32R)
            rb_sb = big.tile([128, N_LT], F32)
            nc.sync.dma_start(out=a_sb[:, 0, :], in_=a[0])
            nc.sync.dma_start(out=a_sb[:, 1, :], in_=a[1])
            nc.sync.dma_start(out=aon_sb[:], in_=aon[:])
            nc.sync.dma_start(out=b_sb[:, 0, :1024], in_=b[0][:, :1024])
            nc.sync.dma_start(out=b_sb[:, 1, :1024], in_=b[1][:, :1024])
            nc.sync.dma_start(out=b_sb[:, 0, 1024:], in_=b[0][:, 1024:])
            nc.sync.dma_start(out=b_sb[:, 1, 1024:], in_=b[1][:, 1024:])
            nc.sync.dma_start(out=bf_sb[:], in_=bfold[:])
            nc.sync.dma_start(out=rb_sb[:], in_=rbias[:])

            for lt in range(N_LT):
                pl = lts[lt]
                l0 = lt * LT
                for st in range(N_ST):
                    s0, stw = ST_OFFS[st], ST_SIZES[st]
                    sim_ps = ps.tile([128, STMAX], F32, tag="sim")
                    for o, w in _chunks(stw):
                        hs = slice(s0 + o, s0 + o + w)
                        ph = slice(o, o + w)
                        nc.tensor.matmul(
                            sim_ps[:pl, ph],
                            a_sb[:, 0, l0 : l0 + pl],
                            b_sb[:, 0, hs],
                            start=True, stop=False,
                        )
                        nc.tensor.matmul(
                            sim_ps[:pl, ph],
                            a_sb[:, 1, l0 : l0 + pl],
                            b_sb[:, 1, hs],
                            start=False, stop=False,
                        )
                        # fold in -lncs/2 via two extra contraction rows (hi+lo)
                        nc.tensor.matmul(
                            sim_ps[:pl, ph],
                            aon_sb[:, l0 : l0 + pl],
                            bf_sb[:, hs],
                            start=False, stop=True,
                        )
                    conf_sb = work.tile([128, STMAX], F32, tag="conf")
                    nc.scalar.activation(
                        conf_sb[:pl, :stw], sim_ps[:pl, :stw], AF.Exp,
                        bias=rb_sb[:pl, lt : lt + 1], scale=2.0,
                    )
                    eng = nc.gpsimd if (lt * N_ST + st) % 2 == 1 else nc.sync
                    eng.dma_start(
                        out=conf_out[l0 : l0 + pl, s0 : s0 + stw],
                        in_=conf_sb[:pl, :stw],
                    )


    _split_excess_waits(nc)
    return nc


_CACHE = {}


def _get_kernels():
    if "k1" not in _CACHE:
        _import_bass()
        _CACHE["k1"] = _build_k1()
        _CACHE["k2"] = _build_k2()
    return _CACHE["k1"], _CACHE["k2"]


def _trunc10(x):
    """Truncate fp32 mantissa to 10 explicit bits (exactly f32r-representable)."""
    xi = np.ascontiguousarray(x, dtype=np.float32).view(np.uint32)
    xi = xi & np.uint32(0xFFFFE000)
    return xi.view(np.float32)


def _border_valid(h, w, bd):
    vh = np.arange(h) >= bd
    vw = np.arange(w) >= bd
    return (vh[:, None] & vw[None, :]).reshape(-1)


def _unpack_cols(arr, n_lt, lts):
    """[128, n_lt] column-per-ltile layout -> flat [sum(lts)]."""
    return np.concatenate([arr[: lts[lt], lt] for lt in range(n_lt)])


def _kernel_device(feat_c0, feat_c1, h0c, w0c, h1c, w1c):
    """Full computation; must run in a process with axon devices visible."""
    import time as _time
    _tp = [("start", _time.time())]
    k1, k2 = _get_kernels()
    _tp.append(("build", _time.time()))
    lts = _lt_sizes()

    # fold feature scale and temperature into the operands:
    # sim/T = (f0*k) @ (f1*k).T with k = C**-0.5 / sqrt(T)
    k = (C ** -0.5) / np.sqrt(np.float32(TEMPERATURE))
    f0t = np.ascontiguousarray(
        (feat_c0 * k).astype(np.float32).transpose(0, 2, 1)
    )  # [N, C, L]
    f1t = np.ascontiguousarray(
        (feat_c1 * k).astype(np.float32).transpose(0, 2, 1)
    )  # [N, C, S]

    a_maps = []
    for c in range(NCORES):
        bidx, j = divmod(c, 4)
        ash = f0t[bidx, :, j * SHARD : (j + 1) * SHARD]  # [C, SHARD]
        a_maps.append(np.ascontiguousarray(ash.reshape(2, 128, SHARD)))
    b_arrs = [np.ascontiguousarray(f1t[bidx].reshape(2, 128, S)) for bidx in range(N)]
    ones128 = np.ones((128, 1), np.float32)

    in_maps1 = [
        {"a": a_maps[c], "b": b_arrs[c // 4], "ones128": ones128}
        for c in range(NCORES)
    ]
    _tp.append(("prep1", _time.time()))
    r1 = run_bass_kernel_spmd(k1, in_maps1, core_ids=list(range(NCORES))).results
    _tp.append(("k1", _time.time()))

    # host combine: rowsum per core (exclusive rows), colsum across the 4 shards
    rowsum = np.empty((N, L), np.float64)
    for c in range(NCORES):
        bidx, j = divmod(c, 4)
        rs = r1[c]["rs_out"].astype(np.float64).reshape(128, N_LT, N_ST).sum(axis=2)
        rowsum[bidx, j * SHARD : (j + 1) * SHARD] = _unpack_cols(rs, N_LT, lts)
    colsum = np.zeros((N, S), np.float64)
    for c in range(NCORES):
        bidx = c // 4
        colsum[bidx] += r1[c]["cs_out"][0].astype(np.float64)

    lnrs = np.log(rowsum)  # [N, L]
    lncs = np.log(colsum)  # [N, S]

    # K2 inputs
    neg_half_lncs = (-0.5 * lncs).astype(np.float32)  # [N, S]
    hi = _trunc10(neg_half_lncs)
    lo = (neg_half_lncs.astype(np.float64) - hi).astype(np.float32)
    bfolds = [np.ascontiguousarray(np.stack([hi[bidx], lo[bidx]])) for bidx in range(N)]
    aon = np.ones((2, SHARD), np.float32)

    in_maps2 = []
    for c in range(NCORES):
        bidx, j = divmod(c, 4)
        nlr = (-lnrs[bidx, j * SHARD : (j + 1) * SHARD]).astype(np.float32)
        rb = np.zeros((128, N_LT), np.float32)
        for lt in range(N_LT):
            rb[: lts[lt], lt] = nlr[lt * LT : lt * LT + lts[lt]]
        in_maps2.append(
            {
                "a": a_maps[c],
                "aon": aon,
                "b": b_arrs[bidx],
                "bfold": bfolds[bidx],
                "rbias": rb,
            }
        )
    _tp.append(("prep2", _time.time()))
    r2 = run_bass_kernel_spmd(k2, in_maps2, core_ids=list(range(NCORES))).results
    _tp.append(("k2", _time.time()))

    conf = np.empty((N, L, S), np.float32)
    conf_max = 0.0
    for c in range(NCORES):
        bidx, j = divmod(c, 4)
        shard = r2[c]["conf_out"]
        conf[bidx, j * SHARD : (j + 1) * SHARD, :] = shard
        conf_max = max(conf_max, float(shard.max()))

    if conf_max <= THR:
        # threshold mask (conf > THR) is all-False -> trivial match outputs.
        match_mask = np.zeros((N, L), dtype=bool)
        j_ids = np.zeros((N, L), dtype=np.int32)
        mconf = np.zeros((N, L), dtype=np.float32)
    else:
        mask = conf > THR
        valid0 = _border_valid(h0c, w0c, BORDER_RM)
        valid1 = _border_valid(h1c, w1c, BORDER_RM)
        mask = mask & valid0[None, :, None] & valid1[None, None, :]
        mask = (
            mask
            & (conf == conf.max(axis=2, keepdims=True))
            & (conf == conf.max(axis=1, keepdims=True))
        )
        j_ids = np.argmax(mask, axis=2).astype(np.int32)
        match_mask = np.any(mask, axis=2)
        mconf = np.take_along_axis(conf, j_ids[..., None].astype(np.int64), axis=2)[
            ..., 0
        ]
        mconf = np.where(match_mask, mconf, 0.0).astype(np.float32)

    _tp.append(("assemble", _time.time()))
    if os.environ.get("KERNEL_TIMING"):
        for (n1, t1), (n2, t2) in zip(_tp, _tp[1:]):
            print(f"  phase {n2}: {t2 - t1:.3f}s", file=sys.stderr)
    return conf, match_mask, j_ids, mconf


def kernel(feat_c0, feat_c1, h0c, w0c, h1c, w1c):
    feat_c0 = np.asarray(feat_c0, dtype=np.float32)
    feat_c1 = np.asarray(feat_c1, dtype=np.float32)
    h0c, w0c, h1c, w1c = int(h0c), int(w0c), int(h1c), int(w1c)
    assert feat_c0.shape == (N, L, C) and feat_c1.shape == (N, S, C)

    if _axon_available():
        return _kernel_device(feat_c0, feat_c1, h0c, w0c, h1c, w1c)

    # The calling process has jax pinned away from the axon platform (e.g.
    # JAX_PLATFORMS=cpu). Run the device work in a clean subprocess.
    with tempfile.TemporaryDirectory() as td:
        in_path = os.path.join(td, "in.npz")
        out_path = os.path.join(td, "out.npz")
        np.savez(in_path, feat_c0=feat_c0, feat_c1=feat_c1,
                 h0c=h0c, w0c=w0c, h1c=h1c, w1c=w1c)
        env = dict(os.environ)
        env.pop("JAX_PLATFORMS", None)
        subprocess.run(
            [sys.executable, os.path.abspath(__file__), "--device-worker",
             in_path, out_path],
            check=True, env=env,
        )
        with np.load(out_path) as z:
            return (z["conf"], z["match_mask"], z["j_ids"], z["mconf"])


if __name__ == "__main__" and len(sys.argv) >= 4 and sys.argv[1] == "--device-worker":
    with np.load(sys.argv[2]) as z:
        _args = (z["feat_c0"], z["feat_c1"],
                 int(z["h0c"]), int(z["w0c"]), int(z["h1c"]), int(z["w1c"]))
    _conf, _mm, _ji, _mc = _kernel_device(*_args)
    np.savez(sys.argv[3], conf=_conf, match_mask=_mm, j_ids=_ji, mconf=_mc)


# revision 19
# speedup vs baseline: 37642.9468x; 1.0853x over previous
"""CoarseMatching dual-softmax kernel for 8 Trainium2 NeuronCores.

Computes conf = softmax_L(sim) * softmax_S(sim) for sim = (f0*s)@(f1*s).T / T,
plus threshold/mutual-NN match extraction, matching reference.py semantics.

Sharding: core c -> (batch b = c//4, L-rows [j*1200,(j+1)*1200) for j = c%4).
Two SPMD launches:
  K1: per-core row sums (free via ACT accum) + column partial sums of exp(sim).
  K2: conf = exp(2*sim - ln(rowsum) - ln(colsum)) streamed to HBM.
Cross-core reduction (colsum over the 4 row-shards of a batch) happens on host
between launches. conf > THR detection is exact: ACT accum gives per-row block
sums of conf; conf>0 so any element > THR forces its block sum > THR. The
(practically unreachable) fallback computes the mask from conf in numpy with
reference semantics.
"""
import os
import subprocess
import sys
import tempfile

import numpy as np

sys.path.insert(0, "/opt/trn_rl_repo")


def _axon_available():
    """True if this process can reach the axon-tunneled NeuronCores."""
    try:
        import jax

        return any(d.platform in ("axon", "neuron") for d in jax.devices())
    except Exception:
        return False


def _import_bass():
    global bass, mybir, tile, run_bass_kernel_spmd, F32, F32R, AF
    import concourse.bass as bass
    import concourse.mybir as mybir
    import concourse.tile as tile
    from concourse.bass_utils import run_bass_kernel_spmd

    F32 = mybir.dt.float32
    F32R = mybir.dt.float32r
    AF = mybir.ActivationFunctionType

# problem constants (hardcoded per harness contract)
N, L, S, C = 2, 4800, 4800, 256
TEMPERATURE = 0.1
THR = 0.2
BORDER_RM = 2

NCORES = 8
SHARD = L // 4          # 1200 rows per core
LT = 128                # partition tile
N_LT = (SHARD + LT - 1) // LT   # 10 (9 full + 48)
# S-axis tiles: PSUM-bank-aligned (bank = 512 fp32). Each matmul chunk must
# stay inside one bank, so tiles are split at 512-element boundaries.
ST_SIZES = [1024, 1024, 1024, 1024, 704]
ST_OFFS = [0, 1024, 2048, 3072, 4096]
N_ST = len(ST_SIZES)
STMAX = 1024

def _chunks(width):
    out, o = [], 0
    while o < width:
        c = min(512, width - o)
        out.append((o, c))
        o += c
    return out

_MAXW = 1  # this toolchain's walrus rejects >1 sync wait on most instructions


def _split_excess_waits(nc):
    """Move excess per-instruction sem waits onto same-engine no-op carriers."""
    if os.environ.get("KERNEL_NO_WAIT_SPLIT"):
        return 0
    cnt = 0
    for f in nc.m.functions:
        for bb in f.blocks:
            il = bb.instructions
            i = 0
            while i < len(il):
                ins = il[i]
                si = ins.sync_info
                lim = 0 if isinstance(ins, mybir.InstMatmult) else _MAXW
                if si is not None and si.on_wait and len(si.on_wait) > lim:
                    waits = list(si.on_wait)
                    si.on_wait = waits[:lim]
                    excess = waits[lim:]
                    for j in range(0, len(excess), _MAXW):
                        cnt += 1
                        nop = mybir.InstNoOp(name=f"waitcarrier_{cnt}", ins=[], outs=[])
                        nop.engine = ins.engine
                        nop.sync_info = mybir.SyncInfo(
                            on_wait=excess[j : j + _MAXW], on_update=[]
                        )
                        il.insert(i, nop)
                        i += 1
                i += 1
    return cnt


def _lt_sizes():
    sizes = []
    for lt in range(N_LT):
        sizes.append(min(LT, SHARD - lt * LT))
    return sizes


def _build_k1():
    """Stats pass: rowsum[SHARD], colsum_part[S] of exp(sim) over local rows."""
    nc = bass.Bass("TRN2")
    a = nc.dram_tensor("a", [C // 128, 128, SHARD], F32R, kind="ExternalInput")
    b = nc.dram_tensor("b", [C // 128, 128, S], F32R, kind="ExternalInput")
    ones128 = nc.dram_tensor("ones128", [128, 1], F32R, kind="ExternalInput")
    rs_out = nc.dram_tensor("rs_out", [128, N_LT * N_ST], F32, kind="ExternalOutput")
    cs_out = nc.dram_tensor("cs_out", [1, S], F32, kind="ExternalOutput")

    lts = _lt_sizes()
    with tile.TileContext(nc) as tc:
        with tc.tile_pool(name="big", bufs=1) as big, \
             tc.tile_pool(name="work", bufs=6) as work, \
             tc.tile_pool(name="ps", bufs=3, space="PSUM") as ps, \
             tc.tile_pool(name="csps", bufs=1, space="PSUM") as csps:
            a_sb = big.tile([128, 2, SHARD], F32R)
            b_sb = big.tile([128, 2, S], F32R)
            ones_sb = big.tile([128, 1], F32R)
            rs_sb = big.tile([128, N_LT * N_ST], F32)
            cs_sb = big.tile([1, S], F32)
            nc.sync.dma_start(out=a_sb[:, 0, :], in_=a[0])
            nc.sync.dma_start(out=a_sb[:, 1, :], in_=a[1])
            nc.sync.dma_start(out=b_sb[:, 0, :1024], in_=b[0][:, :1024])
            nc.sync.dma_start(out=b_sb[:, 1, :1024], in_=b[1][:, :1024])
            nc.gpsimd.dma_start(out=b_sb[:, 0, 1024:], in_=b[0][:, 1024:])
            nc.gpsimd.dma_start(out=b_sb[:, 1, 1024:], in_=b[1][:, 1024:])
            nc.sync.dma_start(out=ones_sb[:], in_=ones128[:])
            nc.vector.memset(rs_sb[:], 0.0)
            nc.vector.memset(cs_sb[:], 0.0)

            for st in range(N_ST):
                s0, stw = ST_OFFS[st], ST_SIZES[st]
                cs_ps = csps.tile([1, STMAX], F32, tag="cs")
                for lt in range(N_LT):
                    pl = lts[lt]
                    l0 = lt * LT
                    sim_ps = ps.tile([128, STMAX], F32, tag="sim")
                    for o, w in _chunks(stw):
                        hs = slice(s0 + o, s0 + o + w)
                        ph = slice(o, o + w)
                        nc.tensor.matmul(
                            sim_ps[:pl, ph],
                            a_sb[:, 0, l0 : l0 + pl],
                            b_sb[:, 0, hs],
                            start=True, stop=False,
                        )
                        nc.tensor.matmul(
                            sim_ps[:pl, ph],
                            a_sb[:, 1, l0 : l0 + pl],
                            b_sb[:, 1, hs],
                            start=False, stop=True,
                        )
                    t_sb = work.tile([128, STMAX], F32R, tag="t")
                    idx = lt * N_ST + st
                    nc.scalar.activation(
                        t_sb[:pl, :stw], sim_ps[:pl, :stw], AF.Exp, scale=1.0,
                    )
                    nc.vector.reduce_sum(
                        rs_sb[:pl, idx : idx + 1],
                        t_sb[:pl, :stw].bitcast(F32),
                        axis=mybir.AxisListType.X,
                    )
                    for o, w in _chunks(stw):
                        ph = slice(o, o + w)
                        nc.tensor.matmul(
                            cs_ps[:1, ph],
                            ones_sb[:pl, :],
                            t_sb[:pl, ph],
                            start=(lt == 0), stop=(lt == N_LT - 1),
                        )
                nc.vector.tensor_copy(cs_sb[:1, s0 : s0 + stw], cs_ps[:1, :stw])

            nc.sync.dma_start(out=rs_out[:, :], in_=rs_sb[:])
            nc.sync.dma_start(out=cs_out[:, :], in_=cs_sb[:])

    _split_excess_waits(nc)
    return nc


def _build_k2():
    """Output pass: conf tiles = exp(2*sim - lnrs - lncs), plus block row sums."""
    nc = bass.Bass("TRN2")
    a = nc.dram_tensor("a", [C // 128, 128, SHARD], F32R, kind="ExternalInput")
    aon = nc.dram_tensor("aon", [2, SHARD], F32R, kind="ExternalInput")   # ones rows
    b = nc.dram_tensor("b", [C // 128, 128, S], F32R, kind="ExternalInput")
    bfold = nc.dram_tensor("bfold", [2, S], F32R, kind="ExternalInput")   # -lncs/2 hi/lo
    rbias = nc.dram_tensor("rbias", [128, N_LT], F32, kind="ExternalInput")  # -lnrs
    conf_out = nc.dram_tensor("conf_out", [SHARD, S], F32, kind="ExternalOutput")

    lts = _lt_sizes()
    with tile.TileContext(nc) as tc:
        with tc.tile_pool(name="big", bufs=1) as big, \
             tc.tile_pool(name="work", bufs=6) as work, \
             tc.tile_pool(name="ps", bufs=3, space="PSUM") as ps:
            a_sb = big.tile([128, 2, SHARD], F32R)
            aon_sb = big.tile([2, SHARD], F32R)
            b_sb = big.tile([128, 2, S], F32R)
            bf_sb = big.tile([2, S], F32R)
            rb_sb = big.tile([128, N_LT], F32)
            nc.sync.dma_start(out=a_sb[:, 0, :], in_=a[0])
            nc.sync.dma_start(out=a_sb[:, 1, :], in_=a[1])
            nc.sync.dma_start(out=aon_sb[:], in_=aon[:])
            nc.sync.dma_start(out=b_sb[:, 0, :1024], in_=b[0][:, :1024])
            nc.sync.dma_start(out=b_sb[:, 1, :1024], in_=b[1][:, :1024])
            nc.gpsimd.dma_start(out=b_sb[:, 0, 1024:], in_=b[0][:, 1024:])
            nc.gpsimd.dma_start(out=b_sb[:, 1, 1024:], in_=b[1][:, 1024:])
            nc.sync.dma_start(out=bf_sb[:], in_=bfold[:])
            nc.sync.dma_start(out=rb_sb[:], in_=rbias[:])

            for lt in range(N_LT):
                pl = lts[lt]
                l0 = lt * LT
                for st in range(N_ST):
                    s0, stw = ST_OFFS[st], ST_SIZES[st]
                    sim_ps = ps.tile([128, STMAX], F32, tag="sim")
                    for o, w in _chunks(stw):
                        hs = slice(s0 + o, s0 + o + w)
                        ph = slice(o, o + w)
                        nc.tensor.matmul(
                            sim_ps[:pl, ph],
                            a_sb[:, 0, l0 : l0 + pl],
                            b_sb[:, 0, hs],
                            start=True, stop=False,
                        )
                        nc.tensor.matmul(
                            sim_ps[:pl, ph],
                            a_sb[:, 1, l0 : l0 + pl],
                            b_sb[:, 1, hs],
                            start=False, stop=False,
                        )
                        # fold in -lncs/2 via two extra contraction rows (hi+lo)
                        nc.tensor.matmul(
                            sim_ps[:pl, ph],
                            aon_sb[:, l0 : l0 + pl],
                            bf_sb[:, hs],
                            start=False, stop=True,
                        )
                    conf_sb = work.tile([128, STMAX], F32, tag="conf")
                    nc.scalar.activation(
                        conf_sb[:pl, :stw], sim_ps[:pl, :stw], AF.Exp,
                        bias=rb_sb[:pl, lt : lt + 1], scale=2.0,
                    )
                    eng = nc.gpsimd if (lt * N_ST + st) % 2 == 1 else nc.sync
                    eng.dma_start(
                        out=conf_out[l0 : l0 + pl, s0 : s0 + stw],
                        in_=conf_sb[:pl, :stw],
                    )


    _split_excess_waits(nc)
    return nc


_CACHE = {}


def _get_kernels():
    if "k1" not in _CACHE:
        _import_bass()
        _CACHE["k1"] = _build_k1()
        _CACHE["k2"] = _build_k2()
    return _CACHE["k1"], _CACHE["k2"]


def _trunc10(x):
    """Truncate fp32 mantissa to 10 explicit bits (exactly f32r-representable)."""
    xi = np.ascontiguousarray(x, dtype=np.float32).view(np.uint32)
    xi = xi & np.uint32(0xFFFFE000)
    return xi.view(np.float32)


def _border_valid(h, w, bd):
    vh = np.arange(h) >= bd
    vw = np.arange(w) >= bd
    return (vh[:, None] & vw[None, :]).reshape(-1)


def _unpack_cols(arr, n_lt, lts):
    """[128, n_lt] column-per-ltile layout -> flat [sum(lts)]."""
    return np.concatenate([arr[: lts[lt], lt] for lt in range(n_lt)])


def _kernel_device(feat_c0, feat_c1, h0c, w0c, h1c, w1c):
    """Full computation; must run in a process with axon devices visible."""
    import time as _time
    _tp = [("start", _time.time())]
    k1, k2 = _get_kernels()
    _tp.append(("build", _time.time()))
    lts = _lt_sizes()

    # fold feature scale and temperature into the operands:
    # sim/T = (f0*k) @ (f1*k).T with k = C**-0.5 / sqrt(T)
    k = (C ** -0.5) / np.sqrt(np.float32(TEMPERATURE))
    f0t = np.ascontiguousarray(
        (feat_c0 * k).astype(np.float32).transpose(0, 2, 1)
    )  # [N, C, L]
    f1t = np.ascontiguousarray(
        (feat_c1 * k).astype(np.float32).transpose(0, 2, 1)
    )  # [N, C, S]

    a_maps = []
    for c in range(NCORES):
        bidx, j = divmod(c, 4)
        ash = f0t[bidx, :, j * SHARD : (j + 1) * SHARD]  # [C, SHARD]
        a_maps.append(np.ascontiguousarray(ash.reshape(2, 128, SHARD)))
    b_arrs = [np.ascontiguousarray(f1t[bidx].reshape(2, 128, S)) for bidx in range(N)]
    ones128 = np.ones((128, 1), np.float32)

    in_maps1 = [
        {"a": a_maps[c], "b": b_arrs[c // 4], "ones128": ones128}
        for c in range(NCORES)
    ]
    _tp.append(("prep1", _time.time()))
    r1 = run_bass_kernel_spmd(k1, in_maps1, core_ids=list(range(NCORES))).results
    _tp.append(("k1", _time.time()))

    # host combine: rowsum per core (exclusive rows), colsum across the 4 shards
    rowsum = np.empty((N, L), np.float64)
    for c in range(NCORES):
        bidx, j = divmod(c, 4)
        rs = r1[c]["rs_out"].astype(np.float64).reshape(128, N_LT, N_ST).sum(axis=2)
        rowsum[bidx, j * SHARD : (j + 1) * SHARD] = _unpack_cols(rs, N_LT, lts)
    colsum = np.zeros((N, S), np.float64)
    for c in range(NCORES):
        bidx = c // 4
        colsum[bidx] += r1[c]["cs_out"][0].astype(np.float64)

    lnrs = np.log(rowsum)  # [N, L]
    lncs = np.log(colsum)  # [N, S]

    # K2 inputs
    neg_half_lncs = (-0.5 * lncs).astype(np.float32)  # [N, S]
    hi = _trunc10(neg_half_lncs)
    lo = (neg_half_lncs.astype(np.float64) - hi).astype(np.float32)
    bfolds = [np.ascontiguousarray(np.stack([hi[bidx], lo[bidx]])) for bidx in range(N)]
    aon = np.ones((2, SHARD), np.float32)

    in_maps2 = []
    for c in range(NCORES):
        bidx, j = divmod(c, 4)
        nlr = (-lnrs[bidx, j * SHARD : (j + 1) * SHARD]).astype(np.float32)
        rb = np.zeros((128, N_LT), np.float32)
        for lt in range(N_LT):
            rb[: lts[lt], lt] = nlr[lt * LT : lt * LT + lts[lt]]
        in_maps2.append(
            {
                "a": a_maps[c],
                "aon": aon,
                "b": b_arrs[bidx],
                "bfold": bfolds[bidx],
                "rbias": rb,
            }
        )
    _tp.append(("prep2", _time.time()))
    r2 = run_bass_kernel_spmd(k2, in_maps2, core_ids=list(range(NCORES))).results
    _tp.append(("k2", _time.time()))

    conf = np.empty((N, L, S), np.float32)
    conf_max = 0.0
    for c in range(NCORES):
        bidx, j = divmod(c, 4)
        shard = r2[c]["conf_out"]
        conf[bidx, j * SHARD : (j + 1) * SHARD, :] = shard
        conf_max = max(conf_max, float(shard.max()))

    if conf_max <= THR:
        # threshold mask (conf > THR) is all-False -> trivial match outputs.
        match_mask = np.zeros((N, L), dtype=bool)
        j_ids = np.zeros((N, L), dtype=np.int32)
        mconf = np.zeros((N, L), dtype=np.float32)
    else:
        mask = conf > THR
        valid0 = _border_valid(h0c, w0c, BORDER_RM)
        valid1 = _border_valid(h1c, w1c, BORDER_RM)
        mask = mask & valid0[None, :, None] & valid1[None, None, :]
        mask = (
            mask
            & (conf == conf.max(axis=2, keepdims=True))
            & (conf == conf.max(axis=1, keepdims=True))
        )
        j_ids = np.argmax(mask, axis=2).astype(np.int32)
        match_mask = np.any(mask, axis=2)
        mconf = np.take_along_axis(conf, j_ids[..., None].astype(np.int64), axis=2)[
            ..., 0
        ]
        mconf = np.where(match_mask, mconf, 0.0).astype(np.float32)

    _tp.append(("assemble", _time.time()))
    if os.environ.get("KERNEL_TIMING"):
        for (n1, t1), (n2, t2) in zip(_tp, _tp[1:]):
            print(f"  phase {n2}: {t2 - t1:.3f}s", file=sys.stderr)
    return conf, match_mask, j_ids, mconf


def kernel(feat_c0, feat_c1, h0c, w0c, h1c, w1c):
    feat_c0 = np.asarray(feat_c0, dtype=np.float32)
    feat_c1 = np.asarray(feat_c1, dtype=np.float32)
    h0c, w0c, h1c, w1c = int(h0c), int(w0c), int(h1c), int(w1c)
    assert feat_c0.shape == (N, L, C) and feat_c1.shape == (N, S, C)

    if _axon_available():
        return _kernel_device(feat_c0, feat_c1, h0c, w0c, h1c, w1c)

    # The calling process has jax pinned away from the axon platform (e.g.
    # JAX_PLATFORMS=cpu). Run the device work in a clean subprocess.
    with tempfile.TemporaryDirectory() as td:
        in_path = os.path.join(td, "in.npz")
        out_path = os.path.join(td, "out.npz")
        np.savez(in_path, feat_c0=feat_c0, feat_c1=feat_c1,
                 h0c=h0c, w0c=w0c, h1c=h1c, w1c=w1c)
        env = dict(os.environ)
        env.pop("JAX_PLATFORMS", None)
        subprocess.run(
            [sys.executable, os.path.abspath(__file__), "--device-worker",
             in_path, out_path],
            check=True, env=env,
        )
        with np.load(out_path) as z:
            return (z["conf"], z["match_mask"], z["j_ids"], z["mconf"])


if __name__ == "__main__" and len(sys.argv) >= 4 and sys.argv[1] == "--device-worker":
    with np.load(sys.argv[2]) as z:
        _args = (z["feat_c0"], z["feat_c1"],
                 int(z["h0c"]), int(z["w0c"]), int(z["h1c"]), int(z["w1c"]))
    _conf, _mm, _ji, _mc = _kernel_device(*_args)
    np.savez(sys.argv[3], conf=_conf, match_mask=_mm, j_ids=_ji, mconf=_mc)
